# revision 4
# baseline (speedup 1.0000x reference)
"""Trainium2 Bass kernel: masked dual 9x9 circular convolution (GridCell).

out = where(mask, circ_conv(x, direction_kernel), circ_conv(x, w_stationary))
x: (1,1,4096,4096) f32, kernels: (1,1,9,9) f32, mask: (4096,4096) bool.

Strategy
--------
Spatially shard H across the 8 NeuronCores (512 rows each).  Host-side we
wrap-pad each shard with the 4-row/4-col circular halo, so no device
collectives are needed.

Each 9x9 circular conv runs on the TensorEngine: for a 120-row output slab,
kernel column dw contributes one matmul  psum[120,512] += Band_dw^T @ xwin
with Band_dw a [K=128, M=120] banded matrix whose 9 diagonals hold
kernel[:, dw]; the 9 matmuls accumulate in one PSUM bank.  float32r matmuls
stream at 1 cycle/row (4x faster than plain fp32) with fp32-stored operands.

The mask select is: ScalarE copies the stationary PSUM to SBUF, then
VectorE copy_predicated overwrites with the velocity PSUM where mask != 0.
"""

import sys

if "/opt/trn_rl_repo" not in sys.path:
    sys.path.insert(0, "/opt/trn_rl_repo")

from contextlib import ExitStack

import numpy as np

import concourse.bacc as bacc
import concourse.bass as bass
import concourse.tile as tile
from concourse import mybir
from concourse.bass_utils import run_bass_kernel_spmd

H = W = 4096
KS = 9
PAD = KS // 2                    # 4 halo rows/cols
NCORES = 8
SH = H // NCORES                 # 512 output rows per core
M = 128 - 2 * PAD                # 120 output rows per matmul slab
NT = 512                         # moving free dim = one fp32 PSUM bank
NWT = W // NT
# Last slab re-computes rows 392..479 (free: matmul cost is N cycles
# regardless of M) and stores only rows 480..511.
SLAB_STARTS = (0, 120, 240, 360, SH - M)

F32R = mybir.dt.float32r
F32 = mybir.dt.float32
U8 = mybir.dt.uint8


def _build_program() -> bass.Bass:
    # Bacc (not raw Bass): its compile() pipeline legalizes sync waits
    # (max 1 wait per instruction on TRN2) via event semaphores.
    nc = bacc.Bacc(None)
    xw = nc.declare_dram_parameter(
        "xw", [SH + 2 * PAD, W + 2 * PAD], F32R, isOutput=False)
    maskp = nc.declare_dram_parameter("maskp", [SH, W], U8, isOutput=False)
    bands = nc.declare_dram_parameter(
        "bands", [128, 2 * KS * M], F32R, isOutput=False)
    out = nc.declare_dram_parameter("out", [SH, W], F32, isOutput=True)

    with ExitStack() as ctx:
        tc = ctx.enter_context(tile.TileContext(nc))
        band_pool = ctx.enter_context(tc.tile_pool(name="band", bufs=1))
        x_pool = ctx.enter_context(tc.tile_pool(name="x", bufs=2))
        m_pool = ctx.enter_context(tc.tile_pool(name="m", bufs=2))
        o_pool = ctx.enter_context(tc.tile_pool(name="o", bufs=2))
        p_pool = ctx.enter_context(tc.tile_pool(name="p", bufs=4, space="PSUM"))

        bsb = band_pool.tile([128, 2 * KS * M], F32R)
        nc.sync.dma_start(bsb[:], bands[:])

        for s, ws in enumerate(SLAB_STARTS):
            xt = x_pool.tile([128, W + 2 * PAD], F32R)
            nc.sync.dma_start(xt[:], xw[ws:ws + 128, :])
            mt = m_pool.tile([M, W], U8)
            nc.sync.dma_start(mt[:], maskp[ws:ws + M, :])
            ot = o_pool.tile([M, W], F32)
            for wt in range(NWT):
                pv = p_pool.tile([M, NT], F32)
                ps = p_pool.tile([M, NT], F32)
                for b in range(KS):
                    rhs = xt[:, wt * NT + b: wt * NT + b + NT]
                    nc.tensor.matmul(pv[:], bsb[:, b * M:(b + 1) * M], rhs,
                                     start=(b == 0), stop=(b == KS - 1))
                    nc.tensor.matmul(ps[:], bsb[:, (KS + b) * M:(KS + b + 1) * M],
                                     rhs, start=(b == 0), stop=(b == KS - 1))
                seg = ot[:, wt * NT:(wt + 1) * NT]
                nc.scalar.copy(seg, ps[:])
                nc.vector.copy_predicated(
                    seg, mt[:, wt * NT:(wt + 1) * NT], pv[:])
            if s < len(SLAB_STARTS) - 1:
                nc.sync.dma_start(out[ws:ws + M, :], ot[:])
            else:
                done = 4 * M
                nc.sync.dma_start(out[done:SH, :], ot[done - ws:M, :])
    nc.finalize()
    return nc


_PROGRAM = None


def _get_program() -> bass.Bass:
    global _PROGRAM
    if _PROGRAM is None:
        _PROGRAM = _build_program()
    return _PROGRAM


def _build_bands(k9: np.ndarray) -> np.ndarray:
    b = np.zeros((KS, 128, M), np.float32)
    j = np.arange(M)
    for dw in range(KS):
        for dh in range(KS):
            b[dw, j + dh, j] = k9[dh, dw]
    return b


def _host_prep(x, direction_kernel, w_stationary, mask):
    x2 = np.asarray(x, dtype=np.float32).reshape(H, W)
    kd = np.asarray(direction_kernel, dtype=np.float32).reshape(KS, KS)
    kst = np.asarray(w_stationary, dtype=np.float32).reshape(KS, KS)
    m2 = np.asarray(mask).reshape(H, W).astype(np.uint8)

    bands = np.concatenate([_build_bands(kd), _build_bands(kst)], axis=0)
    bands_packed = np.ascontiguousarray(
        bands.transpose(1, 0, 2).reshape(128, 2 * KS * M))

    xpad = np.pad(x2, PAD, mode="wrap")
    in_maps = []
    for c in range(NCORES):
        base = c * SH
        in_maps.append({
            "xw": np.ascontiguousarray(xpad[base:base + SH + 2 * PAD, :]),
            "maskp": np.ascontiguousarray(m2[base:base + SH, :]),
            "bands": bands_packed,
        })
    return in_maps


def _run(inputs, trace=False):
    nc = _get_program()
    in_maps = _host_prep(**inputs)
    res = run_bass_kernel_spmd(nc, in_maps, list(range(NCORES)), trace=trace)
    shards = [np.asarray(res.results[c]["out"]) for c in range(NCORES)]
    full = np.concatenate(shards, axis=0).reshape(1, 1, H, W)
    return np.ascontiguousarray(full, dtype=np.float32), res


def kernel(**inputs) -> np.ndarray:
    out, _ = _run(inputs, trace=False)
    return out


# revision 5
# speedup vs baseline: 1.1717x; 1.1717x over previous
"""Trainium2 Bass kernel: masked dual 9x9 circular convolution (GridCell).

out = where(mask, circ_conv(x, direction_kernel), circ_conv(x, w_stationary))
x: (1,1,4096,4096) f32, kernels: (1,1,9,9) f32, mask: (4096,4096) bool.

Strategy
--------
Shard W (columns) across the 8 NeuronCores (512 cols each).  Host-side we
wrap-pad each shard with the 4-row/4-col circular halo, so no device
collectives are needed.  Column sharding makes each core's work an exact
grid of 35 x 120-row slabs by one 512-col tile (vs 40 slab-tiles for row
sharding), cutting TensorEngine work by 12%.

Each 9x9 circular conv runs on the TensorEngine: for a 120-row output slab,
kernel column dw contributes one matmul  psum[120,512] += Band_dw^T @ xwin
with Band_dw a [K=128, M=120] banded matrix whose 9 diagonals hold
kernel[:, dw]; the 9 matmuls accumulate in one PSUM bank.  float32r matmuls
stream at 1 cycle/row (4x faster than plain fp32) with fp32-stored operands.

The mask select is: ScalarE copies the stationary PSUM to SBUF, then
VectorE copy_predicated overwrites with the velocity PSUM where mask != 0.
"""

import sys

if "/opt/trn_rl_repo" not in sys.path:
    sys.path.insert(0, "/opt/trn_rl_repo")

from contextlib import ExitStack

import numpy as np

import concourse.bacc as bacc
import concourse.bass as bass
import concourse.tile as tile
from concourse import mybir
from concourse.bass_utils import run_bass_kernel_spmd

H = W = 4096
KS = 9
PAD = KS // 2                    # 4 halo rows/cols
NCORES = 8
SW = W // NCORES                 # 512 output cols per core
M = 128 - 2 * PAD                # 120 output rows per matmul slab
NT = SW                          # moving free dim = one fp32 PSUM bank
NB = 2 * KS * M                  # packed band-matrix width
# 34 aligned slabs + one final slab re-computing rows 3976..4079 (matmul
# cost is N cycles regardless of M) that stores only rows 4080..4095.
SLAB_STARTS = tuple(120 * s for s in range(34)) + (H - M,)

F32R = mybir.dt.float32r
F32 = mybir.dt.float32
U8 = mybir.dt.uint8


def _build_program() -> bass.Bass:
    # Bacc (not raw Bass): its compile() pipeline legalizes sync waits
    # (max 1 wait per instruction on TRN2) via event semaphores.
    nc = bacc.Bacc(None)
    xw = nc.declare_dram_parameter(
        "xw", [H + 2 * PAD, SW + 2 * PAD], F32R, isOutput=False)
    maskp = nc.declare_dram_parameter("maskp", [H, SW], U8, isOutput=False)
    bandsp = nc.declare_dram_parameter("bands", [128, NB], F32R, isOutput=False)
    out = nc.declare_dram_parameter("out", [H, SW], F32, isOutput=True)

    with ExitStack() as ctx:
        tc = ctx.enter_context(tile.TileContext(nc))
        band_pool = ctx.enter_context(tc.tile_pool(name="band", bufs=1))
        x_pool = ctx.enter_context(tc.tile_pool(name="x", bufs=4))
        m_pool = ctx.enter_context(tc.tile_pool(name="m", bufs=4))
        o_pool = ctx.enter_context(tc.tile_pool(name="o", bufs=4))
        p_pool = ctx.enter_context(tc.tile_pool(name="p", bufs=4, space="PSUM"))

        bv = band_pool.tile([128, KS * M], F32R)
        nc.sync.dma_start(bv[:], bandsp[:, :KS * M])
        bs = band_pool.tile([128, KS * M], F32R)
        nc.sync.dma_start(bs[:], bandsp[:, KS * M:])

        for s, ws in enumerate(SLAB_STARTS):
            xt = x_pool.tile([128, SW + 2 * PAD], F32R)
            nc.sync.dma_start(xt[:], xw[ws:ws + 128, :])
            pv = p_pool.tile([M, NT], F32)
            ps = p_pool.tile([M, NT], F32)
            for b in range(KS):
                nc.tensor.matmul(pv[:], bv[:, b * M:(b + 1) * M],
                                 xt[:, b:b + NT],
                                 start=(b == 0), stop=(b == KS - 1))
            for b in range(KS):
                nc.tensor.matmul(ps[:], bs[:, b * M:(b + 1) * M],
                                 xt[:, b:b + NT],
                                 start=(b == 0), stop=(b == KS - 1))
            mt = m_pool.tile([M, NT], U8)
            nc.sync.dma_start(mt[:], maskp[ws:ws + M, :])
            ot = o_pool.tile([M, NT], F32)
            nc.scalar.copy(ot[:], ps[:])
            nc.vector.copy_predicated(ot[:], mt[:], pv[:])
            if s < len(SLAB_STARTS) - 1:
                nc.sync.dma_start(out[ws:ws + M, :], ot[:])
            else:
                done = 120 * 34
                nc.sync.dma_start(out[done:H, :], ot[done - ws:M, :])
    nc.finalize()
    return nc


_PROGRAM = None


def _get_program() -> bass.Bass:
    global _PROGRAM
    if _PROGRAM is None:
        _PROGRAM = _build_program()
    return _PROGRAM


def _build_bands(k9: np.ndarray) -> np.ndarray:
    b = np.zeros((KS, 128, M), np.float32)
    j = np.arange(M)
    for dw in range(KS):
        for dh in range(KS):
            b[dw, j + dh, j] = k9[dh, dw]
    return b


def _host_prep(x, direction_kernel, w_stationary, mask):
    x2 = np.asarray(x, dtype=np.float32).reshape(H, W)
    kd = np.asarray(direction_kernel, dtype=np.float32).reshape(KS, KS)
    kst = np.asarray(w_stationary, dtype=np.float32).reshape(KS, KS)
    m2 = np.asarray(mask).reshape(H, W).astype(np.uint8)

    bands = np.concatenate([_build_bands(kd), _build_bands(kst)], axis=0)
    bands_packed = np.ascontiguousarray(
        bands.transpose(1, 0, 2).reshape(128, NB))

    xpad = np.pad(x2, PAD, mode="wrap")
    in_maps = []
    for c in range(NCORES):
        base = c * SW
        in_maps.append({
            "xw": np.ascontiguousarray(xpad[:, base:base + SW + 2 * PAD]),
            "maskp": np.ascontiguousarray(m2[:, base:base + SW]),
            "bands": bands_packed,
        })
    return in_maps


def _run(inputs, trace=False):
    nc = _get_program()
    in_maps = _host_prep(**inputs)
    res = run_bass_kernel_spmd(nc, in_maps, list(range(NCORES)), trace=trace)
    shards = [np.asarray(res.results[c]["out"]) for c in range(NCORES)]
    full = np.concatenate(shards, axis=1).reshape(1, 1, H, W)
    return np.ascontiguousarray(full, dtype=np.float32), res


def kernel(**inputs) -> np.ndarray:
    out, _ = _run(inputs, trace=False)
    return out
